# revision 10
# baseline (speedup 1.0000x reference)
"""Trainium2 8-core MoE layer kernel (expert-parallel, Bass/Tile).

Contract: kernel(**inputs) takes the full unsharded numpy inputs of the
MoE reference (hidden_states, router_w, w1, b1, w2, b2) and returns the
full [2, 1024, 2048] float32 output. Internally shards across 8
NeuronCores: one expert per core, replicated FFN weights in bf16,
sharded fp32 router with an AllGather of transposed routing rows,
replicated fp16 capacity scan, dispatch via one-hot permutation matmul,
and a column-chunked AllGather + indirect-gather combine overlapped with
the second FFN matmul.
"""
import numpy as np
import ml_dtypes

import concourse.bass as bass
import concourse.mybir as mybir
import concourse.tile as tile

_PATCH_DOC = """Patch TileContext._drain_and_barrier: the stock version stuffs every
outstanding semaphore wait onto one SP Drain instruction; the installed
walrus rejects >1 sync wait per non-EventSemaphore instruction
("Too many sync wait commands"). Split the waits across a chain of SP
nops, then drain/barrier as before."""
import concourse.tile as tile_mod
from concourse.vector_clock import ScopedClock


def _patched_drain_and_barrier(self, tick_clock, wait_clock):
    nc = self.nc
    carrier = nc.sync.nop(nofuse=True, hint="drain_wait_carrier")
    wait_clock.add_sem_waits(
        carrier.ins, ScopedClock({None: tick_clock.global_clock})
    )
    waits = list(carrier.ins.sync_info.on_wait)
    if len(waits) > 1:
        carrier.ins.sync_info.on_wait = waits[:1]
        import bass_rust as _br
        for w in waits[1:]:
            extra = nc.sync.nop(nofuse=True, hint="drain_wait_carrier")
            extra.ins.sync_info = _br.SyncInfo(on_wait=[w], on_update=[])

    nc.sync.drain()
    nc.all_engine_barrier()
    assert self.sems is not None
    popped = nc._tile_sem_poison_stack.pop()
    assert popped is self._sem_poison
    nc.clear_and_free_semaphores(list(self.sems.allocated().values()))
    nc.all_engine_barrier()


def apply():
    tile_mod.TileContext._drain_and_barrier = _patched_drain_and_barrier


import bass_rust as _br


def split_multi_waits(nc):
    """Walrus in this container accepts at most ONE sync wait per
    instruction. Hoist extra waits onto same-engine NoOps inserted
    immediately before the offending instruction."""
    ctr = 0
    for f in nc.m.functions:
        for b in f.blocks:
            insts = b.instructions
            need = any(
                inst.sync_info is not None and len(inst.sync_info.on_wait) > 1
                for inst in insts
            )
            if not need:
                continue
            out = []
            for inst in insts:
                si = inst.sync_info
                if si is not None and len(si.on_wait) > 1:
                    waits = list(si.on_wait)
                    for w in waits[:-1]:
                        nop = mybir.InstNoOp(name=f"I-wsplit-{ctr}", ins=[], outs=[])
                        ctr += 1
                        nop.engine = inst.engine
                        nop.sync_info = _br.SyncInfo(on_wait=[w], on_update=[])
                        out.append(nop)
                    si.on_wait = waits[-1:]
                out.append(inst)
            b.instructions = out
    return ctr


E, TOPK, CAP, H, F, N, NCORES = 8, 2, 512, 2048, 8192, 2048, 8
S = CAP
TT = N // 128                # 16 token tiles
HT = H // 128                # 16 hidden tiles
FT = F // 128                # 64 ffn tiles
NQ = 4                       # AllGather column chunks
QH = H // NQ                 # 512
# F2 column chunks (offset, width): taper the tail so the last AllGather
# is small and the post-compute tail is short.
F2CHUNKS = [(0, 512), (512, 512), (1024, 512), (1536, 256), (1792, 256)]
TOKC = N // NCORES           # 256
ECAPF = float(E * CAP)

f32 = mybir.dt.float32
f16 = mybir.dt.float16
bf16 = mybir.dt.bfloat16
i32 = mybir.dt.int32
AOP = mybir.AluOpType
AFT = mybir.ActivationFunctionType
AX = mybir.AxisListType


def build_moe(nc: bass.Bass):
    xtm = nc.dram_tensor("xtm", [2, 128, H], f32, kind="ExternalInput")
    xr = nc.dram_tensor("xr", [N, H], bf16, kind="ExternalInput")
    rwsb = nc.dram_tensor("rwsb", [128, HT * E], f32, kind="ExternalInput")
    w1T = nc.dram_tensor("w1tt", [FT, 128, HT * 128], bf16, kind="ExternalInput")
    w2T = nc.dram_tensor("w2T", [F, H], bf16, kind="ExternalInput")
    b1t = nc.dram_tensor("b1t", [128, FT], f32, kind="ExternalInput")
    b2r = nc.dram_tensor("b2r", [1, H], f32, kind="ExternalInput")
    cid = nc.dram_tensor("cid", [1, 1], f32, kind="ExternalInput")
    out = nc.dram_tensor("out", [TOKC, H], f32, kind="ExternalOutput")

    rloc = nc.dram_tensor("rloc", [2, TOKC], f32)
    rall = nc.dram_tensor("rall", [2 * NCORES, TOKC], f32, addr_space="Shared")
    hcq = [nc.dram_tensor(f"hc{q}", [S, qw], bf16)
           for q, (off, qw) in enumerate(F2CHUNKS)]
    hga = [nc.dram_tensor(f"hg{q}", [E * CAP + 1, qw], bf16, addr_space="Shared")
           for q, (off, qw) in enumerate(F2CHUNKS)]

    with tile.TileContext(nc, num_cores=NCORES) as tc:
        with tc.tile_pool(name="persist", bufs=1) as persist:
            _body(nc, tc, persist, xtm, xr, rwsb, w1T, w2T, b1t, b2r, cid, out,
                  rloc, rall, hcq, hga)
    return nc


def _body(nc, tc, persist, xtm, xr, rwsb, w1T, w2T, b1t, b2r, cid, out,
          rloc, rall, hcq, hga):
    # ---- persistent tiles ----
    b2b = persist.tile([128, H], f32, tag="b2b")
    cidb = persist.tile([128, 1], f32, tag="cidb")
    b1sb = persist.tile([128, FT], f32, tag="b1sb")
    rws = persist.tile([128, HT * E], f32, tag="rws")
    zrow = persist.tile([1, QH], bf16, tag="zrow")
    iota512 = persist.tile([128, S], f32, tag="iota512")
    iotaP = persist.tile([128, 1], f32, tag="iotaP")
    ident128 = persist.tile([128, 128], f32, tag="ident128")
    e0loc = persist.tile([128, 2], f32, tag="e0loc")
    e1loc = persist.tile([128, 2], f32, tag="e1loc")
    p0loc = persist.tile([128, 2], f32, tag="p0loc")
    p1loc = persist.tile([128, 2], f32, tag="p1loc")
    cum0a = persist.tile([128, TT], f32, tag="cum0a")
    cum1a = persist.tile([128, TT], f32, tag="cum1a")
    e0a = persist.tile([128, TT], f32, tag="e0a")
    e1a = persist.tile([128, TT], f32, tag="e1a")
    ps0all = persist.tile([128, TT], f32, tag="ps0all")
    ps1all = persist.tile([128, TT], f32, tag="ps1all")
    d0i_tiles = [persist.tile([128, 1], i32, name=f"d0i_{t}", tag=f"d0i_{t}") for t in range(2)]
    d1i_tiles = [persist.tile([128, 1], i32, name=f"d1i_{t}", tag=f"d1i_{t}") for t in range(2)]
    w0_t = [persist.tile([128, 1], f32, name=f"w0_{t}", tag=f"w0_{t}") for t in range(2)]
    w1w_t = [persist.tile([128, 1], f32, name=f"w1cm_{t}", tag=f"w1cm_{t}") for t in range(2)]
    ws_t = [persist.tile([128, 1], f32, name=f"ws_{t}", tag=f"ws_{t}") for t in range(2)]

    # router weights first on the scalar queue (router-critical), then xT tiles
    nc.scalar.dma_start(out=rws[:], in_=rwsb[:, :])
    # small persistent loads on gpsimd
    nc.gpsimd.dma_start(out=b2b[:], in_=b2r[0:1, :].partition_broadcast(128).opt())
    nc.gpsimd.dma_start(out=cidb[:], in_=cid[0:1, :].partition_broadcast(128).opt())
    nc.gpsimd.dma_start(out=b1sb[:], in_=b1t[:, :])
    nc.vector.memset(zrow[:], 0.0)
    for q, (off, qw) in enumerate(F2CHUNKS):
        nc.gpsimd.dma_start(out=hga[q][E * CAP:E * CAP + 1, :],
                            in_=zrow[:, 0:qw])
    with tc.tile_pool(name="iota_tmp", bufs=1) as it_p:
        iota512i = it_p.tile([128, S], i32, tag="iota512i")
        nc.gpsimd.iota(iota512i[:], pattern=[[1, S]], base=0, channel_multiplier=0)
        nc.vector.tensor_copy(out=iota512[:], in_=iota512i[:])
        iotaPi = it_p.tile([128, 1], i32, tag="iotaPi")
        nc.gpsimd.iota(iotaPi[:], pattern=[[0, 1]], base=0, channel_multiplier=1)
        nc.vector.tensor_copy(out=iotaP[:], in_=iotaPi[:])
    nc.vector.tensor_scalar(out=ident128[:], in0=iota512[:, 0:128],
                            scalar1=iotaP[:], scalar2=None, op0=AOP.is_equal)

    # resident dispatch lhsT tiles (plain row-major x) — all on sync queue
    xr_pool = tc.tile_pool(name="xr_res", bufs=1)
    xr_res = xr_pool.__enter__()
    xrt = []
    for tt in range(TT):
        xt = xr_res.tile([128, H], bf16, tag=f"xr_{tt}")
        nc.sync.dma_start(out=xt[:], in_=xr[tt * 128:(tt + 1) * 128, :])
        xrt.append(xt)

    # ============ Phase R: sharded router (own 256 tokens, fp32) ============
    with (tc.tile_pool(name="r_x", bufs=2) as r_x,
          tc.tile_pool(name="r_ps", bufs=2, space="PSUM") as r_ps,
          tc.tile_pool(name="r_tp", bufs=2, space="PSUM") as r_tp,
          tc.tile_pool(name="r_sb", bufs=2) as r_sb,
          tc.tile_pool(name="r_rvt", bufs=1) as r_rvt):
        rvT = r_rvt.tile([2, 2 * 128], f32, tag="rvT")
        lsbs, mx8s = [], []
        for tt2 in range(2):
            xt_t = r_x.tile([128, H], f32, tag="xt_t")
            for qq in range(2):
                nc.scalar.dma_start(
                    out=xt_t[:, qq * (H // 2):(qq + 1) * (H // 2)],
                    in_=xtm[tt2, :, qq * (H // 2):(qq + 1) * (H // 2)])
            ps = r_ps.tile([128, E], f32, tag="r_ps")
            for hc in range(HT):
                nc.tensor.matmul(
                    out=ps[:], lhsT=xt_t[:, hc * 128:(hc + 1) * 128],
                    rhs=rws[:, hc * E:(hc + 1) * E],
                    start=(hc == 0), stop=(hc == HT - 1))
            lsb = r_sb.tile([128, E], f32, tag="lsb")
            nc.vector.tensor_copy(out=lsb[:], in_=ps[:])
            # top-2 on raw logits (same argmax as on softmax probs)
            mx8 = r_sb.tile([128, 8], f32, tag="mx8")
            ix8 = r_sb.tile([128, 8], mybir.dt.uint32, tag="ix8")
            nc.vector.max_with_indices(out_max=mx8[:], out_indices=ix8[:],
                                       in_=lsb[:])
            nc.vector.tensor_copy(out=e0loc[:, tt2:tt2 + 1], in_=ix8[:, 0:1])
            nc.vector.tensor_copy(out=e1loc[:, tt2:tt2 + 1], in_=ix8[:, 1:2])
            # transpose the [128, 2] (e0, e1) block to [2, 128] rows via PE
            rvE = r_sb.tile([128, 2], f32, tag="rvE")
            nc.vector.tensor_copy(out=rvE[:], in_=ix8[:, 0:2])
            tp = r_tp.tile([2, 128], f32, tag="tp")
            nc.tensor.matmul(out=tp[:], lhsT=rvE[:], rhs=ident128[:],
                             start=True, stop=True)
            nc.vector.tensor_copy(out=rvT[:, tt2 * 128:(tt2 + 1) * 128],
                                  in_=tp[:])
            lsbs.append(lsb)
            mx8s.append(mx8)
        nc.scalar.dma_start(out=rloc[:, :], in_=rvT[:])
        nc.gpsimd.collective_compute(
            "AllGather", AOP.bypass,
            replica_groups=[list(range(NCORES))],
            ins=[rloc[:, :].opt()],
            outs=[rall[:, :].opt()])
        # softmax weights for the top-2 picks, off the AllGather critical path:
        # p0 = 1/Z, p1 = exp(l1 - l0)/Z with Z = sum(exp(l - l0))
        for tt2 in range(2):
            lsb, mx8 = lsbs[tt2], mx8s[tt2]
            nm = r_sb.tile([128, 1], f32, tag="nm")
            nc.vector.tensor_scalar_mul(nm[:], mx8[:, 0:1], -1.0)
            ex = r_sb.tile([128, E], f32, tag="ex")
            ssum = r_sb.tile([128, 1], f32, tag="ssum")
            nc.scalar.activation(out=ex[:], in_=lsb[:], func=AFT.Exp,
                                 bias=nm[:], scale=1.0, accum_out=ssum[:])
            rcp = r_sb.tile([128, 1], f32, tag="rcp")
            nc.vector.reciprocal(out=rcp[:], in_=ssum[:])
            nc.vector.tensor_copy(out=p0loc[:, tt2:tt2 + 1], in_=rcp[:])
            d10 = r_sb.tile([128, 1], f32, tag="d10")
            nc.vector.scalar_tensor_tensor(out=d10[:], in0=mx8[:, 0:1],
                                           scalar=-1.0, in1=mx8[:, 1:2],
                                           op0=AOP.mult, op1=AOP.add)
            e1x = r_sb.tile([128, 1], f32, tag="e1x")
            nc.scalar.activation(out=e1x[:], in_=d10[:], func=AFT.Exp,
                                 bias=0.0, scale=1.0)
            nc.vector.tensor_tensor(out=p1loc[:, tt2:tt2 + 1], in0=e1x[:],
                                    in1=rcp[:], op=AOP.mult)

    # ============ Phase S: replicated capacity scan (fp16, (k,e)-major) ======
    # k=0 expert rows live at partitions 0..7, k=1 rows at partitions 32..39
    # (engine ops need base partition 0/32/64/96). Rows 8..31 are zeroed
    # padding so full-tile ops stay finite.
    KP = 40
    with (tc.tile_pool(name="scan", bufs=1) as sc,
          tc.tile_pool(name="s_ps", bufs=1, space="PSUM") as s_ps):
        ebK = sc.tile([KP, N], f32, tag="ebK")
        iopk = sc.tile([KP, 1], f32, tag="iopk")
        totsh = sc.tile([KP, 1], f32, tag="totsh")
        i40 = sc.tile([KP, 2 * E], f16, tag="i40")
        nc.vector.memset(ebK[:], 0.0)
        nc.vector.memset(totsh[:], 0.0)
        # iopk = expert id per row (0..7 at partitions 0..7 and 32..39)
        tge32 = sc.tile([KP, 1], f32, tag="tge32")
        nc.vector.tensor_scalar(out=tge32[:], in0=iotaP[0:KP, :], scalar1=31.5,
                                scalar2=None, op0=AOP.is_gt)
        nc.vector.scalar_tensor_tensor(out=iopk[:], in0=tge32[:], scalar=-32.0,
                                       in1=iotaP[0:KP, :], op0=AOP.mult,
                                       op1=AOP.add)
        # i40: rows 0..7 -> cols 0..7, rows 32..39 -> cols 8..15, rest zero
        psel = sc.tile([KP, 1], f32, tag="psel")
        valid = sc.tile([KP, 1], f32, tag="valid")
        nc.vector.scalar_tensor_tensor(out=psel[:], in0=tge32[:], scalar=-24.0,
                                       in1=iotaP[0:KP, :], op0=AOP.mult,
                                       op1=AOP.add)
        nc.vector.tensor_scalar(out=valid[:], in0=iotaP[0:KP, :], scalar1=7.5,
                                scalar2=None, op0=AOP.is_lt)
        nc.vector.tensor_tensor(out=valid[:], in0=valid[:], in1=tge32[:],
                                op=AOP.add)
        nc.vector.tensor_scalar(out=i40[:], in0=iota512[0:KP, 0:2 * E],
                                scalar1=psel[:], scalar2=None, op0=AOP.is_equal)
        nc.vector.tensor_scalar(out=i40[:], in0=i40[:], scalar1=valid[:],
                                scalar2=None, op0=AOP.mult)
        rview = rall[:, :].rearrange("(c a) n -> a c n", a=2)
        nc.gpsimd.dma_start(
            out=ebK[0:E, :].rearrange("p (c n) -> p c n", c=NCORES),
            in_=rview[0:1, :, :].partition_broadcast(E).opt())
        nc.gpsimd.dma_start(
            out=ebK[32:32 + E, :].rearrange("p (c n) -> p c n", c=NCORES),
            in_=rview[1:2, :, :].partition_broadcast(E).opt())
        ohcat = sc.tile([KP, N], f16, tag="ohcat")
        nc.vector.tensor_scalar(out=ohcat[:], in0=ebK[:], scalar1=iopk[:],
                                scalar2=None, op0=AOP.is_equal)
        ones2n = sc.tile([KP, N], f16, tag="ones2n")
        nc.vector.memset(ones2n[:], 1.0)
        cum = sc.tile([KP, N], f16, tag="cum")
        nc.vector.tensor_tensor_scan(out=cum[:], data0=ones2n[:], data1=ohcat[:],
                                     initial=0.0, op0=AOP.mult, op1=AOP.add)
        # k=1 rows continue from the k=0 totals: shift k0 totals down 32
        # partitions via a tiny SBUF->SBUF DMA, then add.
        nc.gpsimd.dma_start(out=totsh[32:32 + E, :], in_=cum[0:E, N - 1:N])
        nc.vector.tensor_scalar(out=cum[32:32 + E, :], in0=cum[32:32 + E, :],
                                scalar1=totsh[32:32 + E, :], scalar2=None,
                                op0=AOP.add)
        ohcum = sc.tile([KP, N], f16, tag="ohcum")
        nc.vector.tensor_tensor(out=ohcum[:], in0=ohcat[:], in1=cum[:],
                                op=AOP.mult)
        # transpose 16 chunks of [40, 128] -> [128, 16] into one PSUM bank
        psT = s_ps.tile([128, 2 * TT * E], f32, tag="psT")
        for tg in range(TT):
            nc.tensor.matmul(out=psT[:, tg * 16:(tg + 1) * 16],
                             lhsT=ohcum[:, tg * 128:(tg + 1) * 128],
                             rhs=i40[:], start=True, stop=True)
        ocT = sc.tile([128, 2 * TT * E], f32, tag="ocT")
        nc.vector.tensor_copy(out=ocT[:], in_=psT[:])
        ov = ocT[:].rearrange("p (t k e) -> p t k e", k=2, e=E)
        # cum per token (sum over expert one-hot; exactly one nonzero)
        nc.vector.tensor_copy(out=cum0a[:], in_=ov[:, :, 0, 0].opt())
        nc.vector.tensor_copy(out=cum1a[:], in_=ov[:, :, 1, 0].opt())
        for e in range(1, E):
            nc.vector.tensor_tensor(out=cum0a[:], in0=cum0a[:],
                                    in1=ov[:, :, 0, e].opt(), op=AOP.add)
            nc.vector.tensor_tensor(out=cum1a[:], in0=cum1a[:],
                                    in1=ov[:, :, 1, e].opt(), op=AOP.add)
        msk = sc.tile([128, 2 * TT * E], f32, tag="msk")
        nc.vector.tensor_scalar(out=msk[:], in0=ocT[:], scalar1=0.5,
                                scalar2=None, op0=AOP.is_ge)
        mv = msk[:].rearrange("p (t k e) -> p t k e", k=2, e=E)
        nc.vector.tensor_copy(out=e0a[:], in_=mv[:, :, 0, 1].opt())
        nc.vector.tensor_copy(out=e1a[:], in_=mv[:, :, 1, 1].opt())
        for e in range(2, E):
            nc.vector.scalar_tensor_tensor(out=e0a[:], in0=mv[:, :, 0, e].opt(),
                                           scalar=float(e), in1=e0a[:],
                                           op0=AOP.mult, op1=AOP.add)
            nc.vector.scalar_tensor_tensor(out=e1a[:], in0=mv[:, :, 1, e].opt(),
                                           scalar=float(e), in1=e1a[:],
                                           op0=AOP.mult, op1=AOP.add)

    # ============ Phase I: token-major index math ============
    with tc.tile_pool(name="imath", bufs=1) as im:
        m0 = im.tile([128, TT], f32, tag="m0")
        m1 = im.tile([128, TT], f32, tag="m1")
        nc.vector.tensor_scalar(out=m0[:], in0=e0a[:], scalar1=cidb[:],
                                scalar2=None, op0=AOP.is_equal)
        nc.vector.tensor_scalar(out=m1[:], in0=e1a[:], scalar1=cidb[:],
                                scalar2=None, op0=AOP.is_equal)
        t0 = im.tile([128, TT], f32, tag="t0")
        t1 = im.tile([128, TT], f32, tag="t1")
        nc.vector.tensor_tensor(out=t0[:], in0=cum0a[:], in1=m0[:], op=AOP.mult)
        nc.vector.tensor_tensor(out=t1[:], in0=cum1a[:], in1=m1[:], op=AOP.mult)
        nc.vector.tensor_scalar_add(ps0all[:], t0[:], -1.0)
        nc.vector.tensor_scalar_add(ps1all[:], t1[:], -1.0)

        # own-token combine prep (tokens owned: global tiles 2c, 2c+1)
        cid2 = im.tile([128, 1], f32, tag="cid2")
        nc.vector.tensor_scalar_mul(cid2[:], cidb[:], 2.0)
        for t2 in range(2):
            msko = im.tile([128, TT], f32, tag=f"msko_{t2}")
            if t2 == 0:
                nc.vector.tensor_scalar(out=msko[:], in0=iota512[:, 0:TT],
                                        scalar1=cid2[:], scalar2=None,
                                        op0=AOP.is_equal)
            else:
                cid21 = im.tile([128, 1], f32, tag="cid21")
                nc.vector.tensor_scalar_add(cid21[:], cid2[:], 1.0)
                nc.vector.tensor_scalar(out=msko[:], in0=iota512[:, 0:TT],
                                        scalar1=cid21[:], scalar2=None,
                                        op0=AOP.is_equal)
            tmp = im.tile([128, TT], f32, tag="tmp")
            oc0 = im.tile([128, 1], f32, tag="oc0")
            oc1 = im.tile([128, 1], f32, tag="oc1")
            nc.vector.tensor_tensor(out=tmp[:], in0=cum0a[:], in1=msko[:],
                                    op=AOP.mult)
            nc.vector.tensor_reduce(out=oc0[:], in_=tmp[:], op=AOP.add, axis=AX.X)
            nc.vector.tensor_tensor(out=tmp[:], in0=cum1a[:], in1=msko[:],
                                    op=AOP.mult)
            nc.vector.tensor_reduce(out=oc1[:], in_=tmp[:], op=AOP.add, axis=AX.X)
            k0o = im.tile([128, 1], f32, tag="k0o")
            k1o = im.tile([128, 1], f32, tag="k1o")
            nc.vector.tensor_scalar(out=k0o[:], in0=oc0[:], scalar1=float(CAP) + 0.5,
                                    scalar2=None, op0=AOP.is_lt)
            nc.vector.tensor_scalar(out=k1o[:], in0=oc1[:], scalar1=float(CAP) + 0.5,
                                    scalar2=None, op0=AOP.is_lt)
            # d = ((e*CAP + cum - 1) - ECAP)*keep + ECAP
            d0 = im.tile([128, 1], f32, tag="d0")
            d1 = im.tile([128, 1], f32, tag="d1")
            nc.vector.scalar_tensor_tensor(out=d0[:], in0=e0loc[:, t2:t2 + 1],
                                           scalar=float(CAP), in1=oc0[:],
                                           op0=AOP.mult, op1=AOP.add)
            nc.vector.scalar_tensor_tensor(out=d1[:], in0=e1loc[:, t2:t2 + 1],
                                           scalar=float(CAP), in1=oc1[:],
                                           op0=AOP.mult, op1=AOP.add)
            nc.vector.tensor_scalar_add(d0[:], d0[:], -1.0 - ECAPF)
            nc.vector.tensor_scalar_add(d1[:], d1[:], -1.0 - ECAPF)
            nc.vector.tensor_tensor(out=d0[:], in0=d0[:], in1=k0o[:], op=AOP.mult)
            nc.vector.tensor_tensor(out=d1[:], in0=d1[:], in1=k1o[:], op=AOP.mult)
            nc.vector.tensor_scalar_add(d0[:], d0[:], ECAPF)
            nc.vector.tensor_scalar_add(d1[:], d1[:], ECAPF)
            nc.vector.tensor_copy(out=d0i_tiles[t2][:], in_=d0[:])
            nc.vector.tensor_copy(out=d1i_tiles[t2][:], in_=d1[:])
            nc.vector.tensor_tensor(out=w0_t[t2][:], in0=p0loc[:, t2:t2 + 1],
                                    in1=k0o[:], op=AOP.mult)
            nc.vector.tensor_tensor(out=w1w_t[t2][:], in0=p1loc[:, t2:t2 + 1],
                                    in1=k1o[:], op=AOP.mult)
            nc.vector.tensor_tensor(out=ws_t[t2][:], in0=w0_t[t2][:],
                                    in1=w1w_t[t2][:], op=AOP.add)

    # ============ Phase D: P_c build + dispatch matmul ============
    xcT = []
    with tc.tile_pool(name="xc", bufs=1) as xc_pool:
        with (tc.tile_pool(name="dp", bufs=1) as dp,
              tc.tile_pool(name="dp2", bufs=2) as dp2,
              tc.tile_pool(name="d_ps", bufs=2, space="PSUM") as d_ps):
            ptiles = []
            for tt in range(TT):
                oh1 = dp2.tile([128, S], bf16, tag="oh1")
                nc.vector.tensor_scalar(out=oh1[:], in0=iota512[:],
                                        scalar1=ps1all[:, tt:tt + 1],
                                        scalar2=None, op0=AOP.is_equal)
                ptile = dp.tile([128, S], bf16, tag=f"pt_{tt}")
                nc.vector.scalar_tensor_tensor(out=ptile[:], in0=iota512[:],
                                               scalar=ps0all[:, tt:tt + 1],
                                               in1=oh1[:],
                                               op0=AOP.is_equal, op1=AOP.add)
                ptiles.append(ptile)

            for quarter in range(4):
                pd = []
                for hti in range(4):
                    pd_t = d_ps.tile([128, S], f32, tag=f"d_ps_{hti}")
                    pd.append(pd_t)
                for tt in range(TT):
                    for hti in range(4):
                        ht = quarter * 4 + hti
                        nc.tensor.matmul(
                            out=pd[hti][:],
                            lhsT=xrt[tt][:, ht * 128:(ht + 1) * 128],
                            rhs=ptiles[tt][:],
                            start=(tt == 0), stop=(tt == TT - 1))
                for hti in range(4):
                    xt = xc_pool.tile([128, S], bf16,
                                      tag=f"xcT_{quarter * 4 + hti}")
                    nc.scalar.copy(out=xt[:], in_=pd[hti][:])
                    xcT.append(xt)

        # ============ Phase F1 ============
        with tc.tile_pool(name="g", bufs=1) as g_pool:
            g = []
            with (tc.tile_pool(name="f1_w", bufs=8) as f1_w,
                  tc.tile_pool(name="f1_ps", bufs=4, space="PSUM") as f1_ps):
                for ft in range(FT):
                    w1_t = f1_w.tile([128, HT * 128], bf16, tag="w1_t")
                    nc.sync.dma_start(out=w1_t[:], in_=w1T[ft, :, :])
                    ps = f1_ps.tile([128, S], f32, tag="f1_ps")
                    for hc in range(HT):
                        nc.tensor.matmul(
                            out=ps[:], lhsT=w1_t[:, hc * 128:(hc + 1) * 128],
                            rhs=xcT[hc][:],
                            start=(hc == 0), stop=(hc == HT - 1))
                    gt = g_pool.tile([128, S], bf16, tag=f"g_{ft}")
                    nc.scalar.activation(out=gt[:], in_=ps[:], func=AFT.Gelu,
                                         bias=b1sb[:, ft:ft + 1], scale=1.0)
                    g.append(gt)

            # ============ Phase F2 + AllGather + combine per column chunk ====
            with (tc.tile_pool(name="f2_w", bufs=12) as f2_w,
                  tc.tile_pool(name="f2_ps", bufs=2, space="PSUM") as f2_ps,
                  tc.tile_pool(name="f2_o", bufs=4) as f2_o,
                  tc.tile_pool(name="cbp", bufs=2) as cbp):
                for q, (off, qw) in enumerate(F2CHUNKS):
                    psq = []
                    for mt in range(4):
                        psq_t = f2_ps.tile([128, QH], f32, tag=f"f2_ps_{mt}")
                        psq.append(psq_t)
                    for fc in range(FT):
                        w2_t = f2_w.tile([128, QH], bf16, tag="w2_t")
                        nc.sync.dma_start(
                            out=w2_t[:, 0:qw],
                            in_=w2T[fc * 128:(fc + 1) * 128, off:off + qw])
                        for mt in range(4):
                            nc.tensor.matmul(
                                out=psq[mt][:, 0:qw],
                                lhsT=g[fc][:, mt * 128:(mt + 1) * 128],
                                rhs=w2_t[:, 0:qw],
                                start=(fc == 0), stop=(fc == FT - 1))
                    for mt in range(4):
                        ho = f2_o.tile([128, QH], bf16, tag="ho")
                        nc.scalar.copy(out=ho[:, 0:qw], in_=psq[mt][:, 0:qw])
                        nc.gpsimd.dma_start(out=hcq[q][mt * 128:(mt + 1) * 128, :],
                                            in_=ho[:, 0:qw])
                    nc.gpsimd.collective_compute(
                        "AllGather", AOP.bypass,
                        replica_groups=[list(range(NCORES))],
                        ins=[hcq[q][:, :].opt()],
                        outs=[hga[q][0:E * CAP, :].opt()])
                    for tt2 in range(2):
                        g0 = cbp.tile([128, QH], bf16, tag="g0")
                        g1 = cbp.tile([128, QH], bf16, tag="g1")
                        nc.gpsimd.indirect_dma_start(
                            out=g0[:, 0:qw], out_offset=None, in_=hga[q][:, :],
                            in_offset=bass.IndirectOffsetOnAxis(
                                ap=d0i_tiles[tt2][:, :1], axis=0))
                        nc.gpsimd.indirect_dma_start(
                            out=g1[:, 0:qw], out_offset=None, in_=hga[q][:, :],
                            in_offset=bass.IndirectOffsetOnAxis(
                                ap=d1i_tiles[tt2][:, :1], axis=0))
                        a0 = cbp.tile([128, QH], f32, tag="a0")
                        nc.vector.tensor_scalar_mul(a0[:, 0:qw], g0[:, 0:qw],
                                                    w0_t[tt2][:])
                        a1 = cbp.tile([128, QH], f32, tag="a1")
                        nc.vector.scalar_tensor_tensor(
                            out=a1[:, 0:qw], in0=g1[:, 0:qw],
                            scalar=w1w_t[tt2][:], in1=a0[:, 0:qw],
                            op0=AOP.mult, op1=AOP.add)
                        a2 = cbp.tile([128, QH], f32, tag="a2")
                        nc.vector.scalar_tensor_tensor(
                            out=a2[:, 0:qw], in0=b2b[:, off:off + qw],
                            scalar=ws_t[tt2][:], in1=a1[:, 0:qw],
                            op0=AOP.mult, op1=AOP.add)
                        nc.gpsimd.dma_start(
                            out=out[tt2 * 128:(tt2 + 1) * 128, off:off + qw],
                            in_=a2[:, 0:qw])
    xr_pool.__exit__(None, None, None)


# ======================== host-side glue ========================

_CACHE = {}


def _prep_inputs(hidden_states, router_w, w1, b1, w2, b2):
    x = np.asarray(hidden_states, np.float32).reshape(-1, H)
    xT = x.T
    xr = x.astype(ml_dtypes.bfloat16)
    w1Tm = np.asarray(w1, np.float32).T.astype(ml_dtypes.bfloat16)
    w2Tm = np.asarray(w2, np.float32).T.astype(ml_dtypes.bfloat16)
    w1tt = np.ascontiguousarray(
        w1Tm.reshape(16, 128, 64, 128).transpose(2, 1, 0, 3)).reshape(64, 128, 2048)
    rwsT = np.asarray(router_w, np.float32).T  # [H, E]
    rwsb = np.ascontiguousarray(
        rwsT.reshape(HT, 128, E).transpose(1, 0, 2)).reshape(128, HT * E)
    base = {
        "xr": np.ascontiguousarray(xr),
        "w1tt": w1tt,
        "rwsb": rwsb,
        "w2T": np.ascontiguousarray(w2Tm),
        "b1t": np.ascontiguousarray(np.asarray(b1, np.float32).reshape(FT, 128).T),
        "b2r": np.asarray(b2, np.float32).reshape(1, H),
    }
    xtmf = np.ascontiguousarray(
        xT.reshape(16, 128, 16, 128).transpose(2, 1, 0, 3)).reshape(16, 128, 2048)
    ins = []
    for c in range(NCORES):
        m = dict(base)
        m["xtm"] = np.ascontiguousarray(xtmf[2 * c:2 * c + 2])
        m["cid"] = np.full((1, 1), float(c), np.float32)
        ins.append(m)
    return ins


def _get_nc():
    if "nc" not in _CACHE:
        apply()  # tile drain patch
        nc = bass.Bass(num_devices=NCORES)
        build_moe(nc)
        split_multi_waits(nc)
        _CACHE["nc"] = nc
    return _CACHE["nc"]


def kernel(hidden_states, router_w, w1, b1, w2, b2):
    from concourse.bass_utils import run_bass_kernel_spmd

    orig_shape = np.asarray(hidden_states).shape
    nc = _get_nc()
    ins = _prep_inputs(hidden_states, router_w, w1, b1, w2, b2)
    res = run_bass_kernel_spmd(nc, ins, core_ids=list(range(NCORES)))
    full = np.concatenate([res.results[c]["out"] for c in range(NCORES)], axis=0)
    return full.reshape(orig_shape).astype(np.float32)
